# revision 6
# baseline (speedup 1.0000x reference)
"""TRN2 Bass kernel for nn_Block_35820027249020 (spiking transformer block).

Sharding: data-parallel over batch B=8 -> one sample per NeuronCore, the full
block computed per core, no collectives.

Precision strategy (results fp32-accurate, ~1e-6 relative):
  - conv1x1 on binary spikes: W = Whi(fp16) + Wlo(fp16), 2 accumulating PE
    passes (PE multiplies fp16 subnormals exactly; products of fp16 x {0,1}
    are exact in the fp32 PSUM accumulator).
  - conv1x1 on continuous input z: z split into zhi+zlo fp16, 3 passes
    (Whi@zhi + Whi@zlo + Wlo@zhi).
  - attention on binary q,k,v: plain fp16, bit-exact integer arithmetic.
  - depthwise 3x3, BN, LIF: fp32 on DVE/ACT/GPSIMD.

LIF recurrences use a doubled accumulator S = 2*mem: S = 0.5*R + u,
spike = (S >= 2*thresh), R = S*[S < 2*thresh]; S overwrites R in place.
"""
import sys

sys.path.insert(0, "/opt/trn_rl_repo")

import numpy as np
from contextlib import ExitStack

T, C, N, HID, HEADS = 4, 512, 256, 2048, 8
KC = C // 128          # 4 contraction chunks of 128
MC = C // 128          # 4 output chunks of 128
MH = HID // 128        # 16 mlp hidden chunks
EPS = 1e-5

_CACHE = {}


# ------------------------------------------------------------- wait post-pass
def _split_multi_waits(nc):
    """HW instructions encode at most ONE sync-wait; hoist extras onto
    same-engine NoOps inserted immediately before the instruction."""
    from concourse import mybir

    for f in nc.m.functions:
        for bb in f.blocks:
            ins = bb.instructions
            if not any(
                i.sync_info is not None and i.sync_info.on_wait
                and len(i.sync_info.on_wait) > 1 for i in ins
            ):
                continue
            out = []
            for inst in ins:
                si = inst.sync_info
                if si is not None and si.on_wait and len(si.on_wait) > 1:
                    waits = list(si.on_wait)
                    for j, w in enumerate(waits[:-1]):
                        out.append(mybir.InstNoOp(
                            name=f"{inst.name}-wsplit{j}",
                            engine=inst.engine,
                            sync_info=mybir.SyncInfo(on_wait=[w], on_update=[]),
                            bass_nofuse=True,
                        ))
                    inst.sync_info = mybir.SyncInfo(
                        on_wait=[waits[-1]], on_update=list(si.on_update or []))
                out.append(inst)
            bb.instructions[:] = out


# ---------------------------------------------------------------- host prep
def _bn_fold(p):
    g = np.asarray(p["gamma"], np.float32)
    b = np.asarray(p["beta"], np.float32)
    m = np.asarray(p["mean"], np.float32)
    v = np.asarray(p["var"], np.float32)
    s = g / np.sqrt(v + np.float32(EPS))
    return s, b - m * s


def _pp(a):  # (C,) -> (128, C//128): [p, mc] = a[mc*128+p]
    a = np.asarray(a, np.float32)
    return np.ascontiguousarray(a.reshape(-1, 128).T)


def _lhsT(w):
    """w: (O, K) -> (128, (K/128)*O): [p, kc*O + m] = w[m, kc*128+p]"""
    wT = np.asarray(w, np.float32).T  # (K, O)
    K, O = wT.shape
    return np.ascontiguousarray(
        wT.reshape(K // 128, 128, O).transpose(1, 0, 2).reshape(128, -1))


def _lhsT_momajor(w):
    """w: (O, K) -> (128, (O/128)*K): [p, mo*K + kc*128 + j] = w[mo*128+j, kc*128+p]
    (per-mo contiguous slices for streaming)"""
    wT = np.asarray(w, np.float32).T  # (K, O)
    K, O = wT.shape
    a = wT.reshape(K // 128, 128, O // 128, 128)   # [kc, p, mo, j]
    a = a.transpose(1, 2, 0, 3)                     # [p, mo, kc, j]
    return np.ascontiguousarray(a.reshape(128, -1))


def _split16(a):
    hi = a.astype(np.float16)
    lo = (a - hi.astype(np.float32)).astype(np.float16)
    return hi, lo


def _prep_common(params):
    attn, mlp = params["attn"], params["mlp"]
    common = {}
    for name in ("q", "k", "v", "proj"):
        p = attn[name]
        w1h, w1l = _split16(_lhsT(np.asarray(p["w1"], np.float32)))
        wph, wpl = _split16(_lhsT(np.asarray(p["wpw"], np.float32)))
        s_p, b_p = _bn_fold(p["bnp"])
        s_bn, b_bn = _bn_fold(p["bn"])
        s_x, b_x = _bn_fold(attn["proj_bn" if name == "proj" else name + "_bn"])
        sC = s_bn * s_x
        bC = b_bn * s_x + b_x
        wdw = np.asarray(p["wdw"], np.float32).reshape(C, 9)
        wdw = np.ascontiguousarray(
            wdw.reshape(MC, 128, 9).transpose(1, 0, 2).reshape(128, MC * 9))
        common.update({
            f"w1h_{name}": w1h, f"w1l_{name}": w1l,
            f"wph_{name}": wph, f"wpl_{name}": wpl,
            f"bnps_{name}": _pp(s_p), f"bnpb_{name}": _pp(b_p),
            f"sC_{name}": _pp(sC), f"bC_{name}": _pp(bC),
            f"wdw_{name}": wdw,
        })
    w1mh, w1ml = _split16(_lhsT_momajor(np.asarray(mlp["w1"], np.float32)))
    w2mh, w2ml = _split16(_lhsT_momajor(np.asarray(mlp["w2"], np.float32)))
    s1, b1 = _bn_fold(mlp["bn1"])
    s2, b2 = _bn_fold(mlp["bn2"])
    common.update({
        "w1mh": w1mh, "w1ml": w1ml, "w2mh": w2mh, "w2ml": w2ml,
        "s1": _pp(s1), "b1": _pp(b1), "s2": _pp(s2), "b2": _pp(b2),
    })
    return common


# ---------------------------------------------------------------- program
def _build_program():
    import concourse.bass as bass
    import concourse.mybir as mybir
    import concourse.tile as tile
    from concourse import masks

    F32 = mybir.dt.float32
    F16 = mybir.dt.float16
    ALU = mybir.AluOpType
    AF = mybir.ActivationFunctionType

    nc = bass.Bass()

    d = {}
    def dp(name, shape, dtype=F32):
        d[name] = nc.declare_dram_parameter(name, list(shape), dtype, isOutput=False)

    dp("x", (T, 128, KC * N))
    for nm in ("q", "k", "v", "proj"):
        dp(f"w1h_{nm}", (128, KC * C), F16)
        dp(f"w1l_{nm}", (128, KC * C), F16)
        dp(f"wph_{nm}", (128, KC * C), F16)
        dp(f"wpl_{nm}", (128, KC * C), F16)
        dp(f"bnps_{nm}", (128, MC))
        dp(f"bnpb_{nm}", (128, MC))
        dp(f"sC_{nm}", (128, MC))
        dp(f"bC_{nm}", (128, MC))
        dp(f"wdw_{nm}", (128, MC * 9))
    dp("w1mh", (128, KC * HID), F16)
    dp("w1ml", (128, KC * HID), F16)
    dp("w2mh", (128, MC * HID), F16)
    dp("w2ml", (128, MC * HID), F16)
    dp("s1", (128, MH)); dp("b1", (128, MH))
    dp("s2", (128, MC)); dp("b2", (128, MC))
    d_out = nc.declare_dram_parameter("out", [T, 128, KC * N], F32, isOutput=True)

    with ExitStack() as ctx:
        tc = ctx.enter_context(tile.TileContext(nc))
        cpool = ctx.enter_context(tc.tile_pool(name="consts", bufs=1))
        spool = ctx.enter_context(tc.tile_pool(name="states", bufs=1))
        wpool = ctx.enter_context(tc.tile_pool(name="wstream", bufs=2))
        iop = ctx.enter_context(tc.tile_pool(name="io", bufs=2))
        fpool = ctx.enter_context(tc.tile_pool(name="frames", bufs=2))
        spk = ctx.enter_context(tc.tile_pool(name="spikes", bufs=2))
        wk = ctx.enter_context(tc.tile_pool(name="work", bufs=2))
        hp = ctx.enter_context(tc.tile_pool(name="hsmall", bufs=4))
        pconv = ctx.enter_context(tc.tile_pool(name="psc", bufs=4, space="PSUM"))
        ptr = ctx.enter_context(tc.tile_pool(name="pst", bufs=2, space="PSUM"))
        pkv = ctx.enter_context(tc.tile_pool(name="psk", bufs=1, space="PSUM"))

        def load_const(name, shape, dtype=F32):
            t = cpool.tile(list(shape), dtype, tag=name)
            nc.sync.dma_start(t[:], d[name][:, :])
            return t

        W = {}
        for nm in ("q", "k", "v", "proj"):
            for part in ("w1h", "w1l"):
                W[f"{part}_{nm}"] = load_const(f"{part}_{nm}", (128, KC * C), F16)
            for part in ("bnps", "bnpb", "sC", "bC"):
                W[f"{part}_{nm}"] = load_const(f"{part}_{nm}", (128, MC))
            W[f"wdw_{nm}"] = load_const(f"wdw_{nm}", (128, MC * 9))
        W["s1"] = load_const("s1", (128, MH)); W["b1"] = load_const("b1", (128, MH))
        W["s2"] = load_const("s2", (128, MC)); W["b2"] = load_const("b2", (128, MC))

        ident = cpool.tile([128, 128], F16, tag="ident")
        masks.make_identity(nc, ident[:])

        R = {}
        for snm, cols in (("x", KC * N), ("q", KC * N), ("k", KC * N),
                          ("v", KC * N), ("a", KC * N), ("m1", KC * N),
                          ("m2", MH * N)):
            R[snm] = spool.tile([128, cols], F32, tag=f"R{snm}",
                                name=f"R{snm}")

        def spike_reset(Rt, thr2, out_spike, skip_reset):
            nc.gpsimd.tensor_scalar(
                out=out_spike[:], in0=Rt[:], scalar1=float(thr2), scalar2=None,
                op0=ALU.is_ge)
            if not skip_reset:
                nc.vector.scalar_tensor_tensor(
                    out=Rt[:], in0=Rt[:], scalar=float(thr2), in1=Rt[:],
                    op0=ALU.is_lt, op1=ALU.mult)

        for t in range(T):
            last = (t == T - 1)

            xt = iop.tile([128, KC * N], F32, tag="xt", bufs=1)
            nc.sync.dma_start(xt[:], d["x"][t, :, :])

            # ---- LIF on x -> xl spikes
            xl = spk.tile([128, KC * N], F16, tag="xl")
            if t == 0:
                nc.gpsimd.tensor_scalar(out=xl[:], in0=xt[:], scalar1=2.0,
                                        scalar2=None, op0=ALU.is_ge)
                nc.vector.scalar_tensor_tensor(
                    out=R["x"][:], in0=xt[:], scalar=2.0, in1=xt[:],
                    op0=ALU.is_lt, op1=ALU.mult)
            else:
                nc.vector.scalar_tensor_tensor(
                    out=R["x"][:], in0=R["x"][:], scalar=0.5, in1=xt[:],
                    op0=ALU.mult, op1=ALU.add)
                spike_reset(R["x"], 2.0, xl, skip_reset=last)

            proj_ps = []

            def repconv(nm, rhs_spike, Rt, out_spike, skip_reset):
                """repconv(+folded bn2) -> LIF step into Rt/out_spike,
                or stash wpw psum tiles into proj_ps when Rt is None."""
                # conv1 (2-pass fp16 spike matmul) -> frame with bnp fused
                frame = fpool.tile([128, MC * 324], F32, tag="frame")
                w1h, w1l = W[f"w1h_{nm}"], W[f"w1l_{nm}"]
                for mc in range(MC):
                    ps = pconv.tile([128, N], F32, tag="psc")
                    first = True
                    for wt in (w1h, w1l):
                        for kc in range(KC):
                            nc.tensor.matmul(
                                ps[:],
                                wt[:, kc * C + mc * 128: kc * C + mc * 128 + 128],
                                rhs_spike[:, kc * N:(kc + 1) * N],
                                start=first, stop=(wt is w1l and kc == KC - 1))
                            first = False
                    fr = frame[:, mc * 324:(mc + 1) * 324].rearrange(
                        "p (h w) -> p h w", h=18)
                    nc.scalar.activation(
                        fr[:, 0:18:17, :], fr[:, 0:18:17, :], AF.Identity,
                        bias=W[f"bnpb_{nm}"][:, mc:mc + 1], scale=0.0)
                    nc.scalar.activation(
                        fr[:, 1:17, 0:18:17], fr[:, 1:17, 0:18:17], AF.Identity,
                        bias=W[f"bnpb_{nm}"][:, mc:mc + 1], scale=0.0)
                    nc.scalar.activation(
                        fr[:, 1:17, 1:17], ps[:], AF.Identity,
                        bias=W[f"bnpb_{nm}"][:, mc:mc + 1],
                        scale=W[f"bnps_{nm}"][:, mc:mc + 1])

                # depthwise 3x3: 9 shifted multiply-accumulates per chunk
                acc = wk.tile([128, MC * N], F32, tag="acc")
                for mc in range(MC):
                    fr = frame[:, mc * 324:(mc + 1) * 324].rearrange(
                        "p (h w) -> p h w", h=18)
                    a_mc = acc[:, mc * N:(mc + 1) * N].rearrange(
                        "p (h w) -> p h w", h=16)
                    eng2 = nc.vector
                    for tap in range(9):
                        di, dj = tap // 3, tap % 3
                        sh = fr[:, di:di + 16, dj:dj + 16]
                        wtap = W[f"wdw_{nm}"][:, mc * 9 + tap:mc * 9 + tap + 1]
                        if tap == 0:
                            nc.scalar.activation(a_mc[:], sh[:], AF.Copy,
                                                 bias=0.0, scale=wtap)
                        else:
                            eng2.scalar_tensor_tensor(
                                out=a_mc[:], in0=sh[:], scalar=wtap,
                                in1=a_mc[:], op0=ALU.mult, op1=ALU.add)

                # split acc -> fp16 hi/lo for exact 3-pass wpw conv
                zh = wk.tile([128, MC * N], F16, tag="zh")
                zl = wk.tile([128, MC * N], F16, tag="zl")
                nc.scalar.copy(zh[:], acc[:])
                nc.vector.tensor_tensor(out=zl[:], in0=acc[:], in1=zh[:],
                                        op=ALU.subtract)

                wph = wpool.tile([128, KC * C], F16, tag="wph")
                wpl = wpool.tile([128, KC * C], F16, tag="wpl")
                nc.sync.dma_start(wph[:], d[f"wph_{nm}"][:, :])
                nc.sync.dma_start(wpl[:], d[f"wpl_{nm}"][:, :])
                for mc in range(MC):
                    ps = pconv.tile([128, N], F32, tag="psc")
                    first = True
                    for wt, zt in ((wph, zh), (wph, zl), (wpl, zh)):
                        for kc in range(KC):
                            nc.tensor.matmul(
                                ps[:],
                                wt[:, kc * C + mc * 128: kc * C + mc * 128 + 128],
                                zt[:, kc * N:(kc + 1) * N],
                                start=first,
                                stop=(wt is wpl and zt is zh and kc == KC - 1))
                            first = False
                    if Rt is None:
                        proj_ps.append(ps)
                    elif t == 0:
                        nc.scalar.activation(
                            Rt[:, mc * N:(mc + 1) * N], ps[:], AF.Identity,
                            bias=W[f"bC_{nm}"][:, mc:mc + 1],
                            scale=W[f"sC_{nm}"][:, mc:mc + 1])
                    else:
                        h = hp.tile([128, N], F32, tag="h256")
                        nc.scalar.activation(
                            h[:], Rt[:, mc * N:(mc + 1) * N], AF.Identity,
                            bias=W[f"bC_{nm}"][:, mc:mc + 1], scale=0.5)
                        nc.vector.scalar_tensor_tensor(
                            out=Rt[:, mc * N:(mc + 1) * N], in0=ps[:],
                            scalar=W[f"sC_{nm}"][:, mc:mc + 1], in1=h[:],
                            op0=ALU.mult, op1=ALU.add)
                if Rt is not None:
                    spike_reset(Rt, 2.0, out_spike, skip_reset)

            sq = spk.tile([128, KC * N], F16, tag="sq", bufs=1)
            sk = spk.tile([128, KC * N], F16, tag="sk", bufs=1)
            sv = spk.tile([128, KC * N], F16, tag="sv", bufs=1)
            repconv("q", xl, R["q"], sq, last)
            repconv("k", xl, R["k"], sk, last)
            repconv("v", xl, R["v"], sv, last)

            # ---- attention
            kT = wk.tile([128, 2 * C], F16, tag="kT", bufs=1)
            vT = wk.tile([128, 2 * C], F16, tag="vT", bufs=1)
            for src, dst in ((sk, kT), (sv, vT)):
                for cc in range(KC):
                    for nb in range(2):
                        pst = ptr.tile([128, 128], F16, tag="pst")
                        nc.tensor.transpose(
                            pst[:],
                            src[:, cc * N + nb * 128: cc * N + nb * 128 + 128],
                            ident[:])
                        nc.scalar.copy(
                            dst[:, nb * C + cc * 128: nb * C + cc * 128 + 128],
                            pst[:])

            pskv = pkv.tile([128, N], F32, tag="pskv")
            for h8 in range(HEADS):
                par, j = h8 % 2, h8 // 2
                for nb in range(2):
                    nc.tensor.matmul(
                        pskv[par * 64:par * 64 + 64, j * 64:j * 64 + 64],
                        kT[:, nb * C + h8 * 64: nb * C + h8 * 64 + 64],
                        vT[:, nb * C + h8 * 64: nb * C + h8 * 64 + 64],
                        start=(nb == 0), stop=(nb == 1),
                        tile_position=(0, par * 64))
            kvs = wk.tile([128, N], F16, tag="kvs", bufs=1)
            nc.scalar.copy(kvs[:], pskv[:])

            if t > 0:
                ha = wk.tile([128, KC * N], F32, tag="ha", bufs=1)
                nc.scalar.activation(ha[:], R["a"][:], AF.Copy, bias=0.0,
                                     scale=0.5)
            sa = spk.tile([128, KC * N], F16, tag="sa", bufs=1)
            for j in range(KC):
                psa = pconv.tile([128, N], F32, tag="psc")
                for par in range(2):
                    nc.tensor.matmul(
                        psa[par * 64:par * 64 + 64, :],
                        kvs[par * 64:par * 64 + 64, j * 64:j * 64 + 64],
                        sq[par * 64:par * 64 + 64, j * N:(j + 1) * N],
                        start=True, stop=True,
                        tile_position=(par * 64, par * 64))
                if t == 0:
                    nc.scalar.activation(R["a"][:, j * N:(j + 1) * N], psa[:],
                                         AF.Copy, bias=0.0, scale=0.125)
                else:
                    nc.vector.scalar_tensor_tensor(
                        out=R["a"][:, j * N:(j + 1) * N], in0=psa[:],
                        scalar=0.125, in1=ha[:, j * N:(j + 1) * N],
                        op0=ALU.mult, op1=ALU.add)
            spike_reset(R["a"], 1.0, sa, last)

            # ---- proj repconv -> x2 = xt + projbn(...)
            repconv("proj", sa, None, None, last)
            x2 = iop.tile([128, KC * N], F32, tag="x2")
            for mc in range(MC):
                xtb = hp.tile([128, N], F32, tag="h256")
                nc.scalar.activation(xtb[:], xt[:, mc * N:(mc + 1) * N],
                                     AF.Identity,
                                     bias=W["bC_proj"][:, mc:mc + 1], scale=1.0)
                nc.vector.scalar_tensor_tensor(
                    out=x2[:, mc * N:(mc + 1) * N], in0=proj_ps[mc][:],
                    scalar=W["sC_proj"][:, mc:mc + 1], in1=xtb[:],
                    op0=ALU.mult, op1=ALU.add)

            # ---- mlp
            y = spk.tile([128, KC * N], F16, tag="y", bufs=1)
            if t == 0:
                nc.gpsimd.tensor_scalar(out=y[:], in0=x2[:], scalar1=2.0,
                                        scalar2=None, op0=ALU.is_ge)
                nc.vector.scalar_tensor_tensor(
                    out=R["m1"][:], in0=x2[:], scalar=2.0, in1=x2[:],
                    op0=ALU.is_lt, op1=ALU.mult)
            else:
                nc.vector.scalar_tensor_tensor(
                    out=R["m1"][:], in0=R["m1"][:], scalar=0.5, in1=x2[:],
                    op0=ALU.mult, op1=ALU.add)
                spike_reset(R["m1"], 2.0, y, skip_reset=last)

            zs = wk.tile([128, MH * N], F16, tag="zs", bufs=1)
            for mo in range(MH):
                w1h_mo = wpool.tile([128, KC * 128], F16, tag="w1mh", bufs=3)
                w1l_mo = wpool.tile([128, KC * 128], F16, tag="w1ml", bufs=3)
                nc.sync.dma_start(w1h_mo[:], d["w1mh"][:, mo * KC * 128:
                                                       (mo + 1) * KC * 128])
                nc.sync.dma_start(w1l_mo[:], d["w1ml"][:, mo * KC * 128:
                                                       (mo + 1) * KC * 128])
                ps = pconv.tile([128, N], F32, tag="psc")
                first = True
                for wt in (w1h_mo, w1l_mo):
                    for kc in range(KC):
                        nc.tensor.matmul(
                            ps[:],
                            wt[:, kc * 128: kc * 128 + 128],
                            y[:, kc * N:(kc + 1) * N],
                            start=first, stop=(wt is w1l_mo and kc == KC - 1))
                        first = False
                if t == 0:
                    nc.scalar.activation(
                        R["m2"][:, mo * N:(mo + 1) * N], ps[:], AF.Identity,
                        bias=W["b1"][:, mo:mo + 1], scale=W["s1"][:, mo:mo + 1])
                else:
                    h = hp.tile([128, N], F32, tag="h256")
                    nc.scalar.activation(
                        h[:], R["m2"][:, mo * N:(mo + 1) * N], AF.Identity,
                        bias=W["b1"][:, mo:mo + 1], scale=0.5)
                    nc.vector.scalar_tensor_tensor(
                        out=R["m2"][:, mo * N:(mo + 1) * N], in0=ps[:],
                        scalar=W["s1"][:, mo:mo + 1], in1=h[:],
                        op0=ALU.mult, op1=ALU.add)
            spike_reset(R["m2"], 2.0, zs, last)

            # ---- mlp W2 (streamed per output chunk) + bn2 + residual
            ot = iop.tile([128, KC * N], F32, tag="ot", bufs=1)
            for mo in range(MC):
                w2h = wpool.tile([128, MH * 128], F16, tag="w2h")
                w2l = wpool.tile([128, MH * 128], F16, tag="w2l")
                nc.sync.dma_start(w2h[:], d["w2mh"][:, mo * MH * 128:
                                                    (mo + 1) * MH * 128])
                nc.sync.dma_start(w2l[:], d["w2ml"][:, mo * MH * 128:
                                                    (mo + 1) * MH * 128])
                ps = pconv.tile([128, N], F32, tag="psc")
                first = True
                for wt in (w2h, w2l):
                    for kc in range(MH):
                        nc.tensor.matmul(
                            ps[:],
                            wt[:, kc * 128: kc * 128 + 128],
                            zs[:, kc * N:(kc + 1) * N],
                            start=first, stop=(wt is w2l and kc == MH - 1))
                        first = False
                x2b = hp.tile([128, N], F32, tag="h256")
                nc.scalar.activation(x2b[:], x2[:, mo * N:(mo + 1) * N],
                                     AF.Identity,
                                     bias=W["b2"][:, mo:mo + 1], scale=1.0)
                nc.vector.scalar_tensor_tensor(
                    out=ot[:, mo * N:(mo + 1) * N], in0=ps[:],
                    scalar=W["s2"][:, mo:mo + 1], in1=x2b[:],
                    op0=ALU.mult, op1=ALU.add)
            nc.sync.dma_start(d_out[t, :, :], ot[:])

    _split_multi_waits(nc)
    return nc


# ---------------------------------------------------------------- entry
def kernel(x, params):
    from concourse.bass_utils import run_bass_kernel_spmd

    if "nc" not in _CACHE:
        _CACHE["nc"] = _build_program()
    nc = _CACHE["nc"]

    x = np.asarray(x, np.float32)
    common = _prep_common(params)
    B = x.shape[1]
    in_maps = []
    for b in range(B):
        xb = x[:, b].reshape(T, KC, 128, N).transpose(0, 2, 1, 3)
        m = dict(common)
        m["x"] = np.ascontiguousarray(xb.reshape(T, 128, KC * N))
        in_maps.append(m)

    res = run_bass_kernel_spmd(nc, in_maps, list(range(8)))

    outs = []
    for b in range(B):
        ob = res.results[b]["out"]
        ob = ob.reshape(T, 128, KC, N).transpose(0, 2, 1, 3).reshape(T, C, 16, 16)
        outs.append(ob)
    return np.stack(outs, axis=1).astype(np.float32)
